# revision 25
# baseline (speedup 1.0000x reference)
"""Causal self-attention (b=2, t=2048, d=1024, h=16) on 8 trn2 NeuronCores.

Sharding: core c handles batch c//4 and the 4 heads 4*(c%4)..4*(c%4)+3
(data parallel over batch x tensor parallel over heads). Each core
computes x @ w_qkv for its head-slice, causal attention for its heads,
and a partial out-projection  y_heads @ w_out[head_rows]; the host sums
the 4 partial outputs per batch (the tensor-parallel all-reduce).

Layout/perf notes:
  x is transposed on the HOST (f32) so the kernel does plain contiguous
  DMAs into f32r tiles (no DMA-transpose, no hi/lo bf16 split, no DVE
  merge). Weights are host-swizzled to [128, chunks, n].
  Input DMAs are spread across the Sync/Scalar/GpSimd queues so issue
  (~1.3us each) does not serialize the head of the kernel.
  qT, kT [dh, t] f32r per head-pair (128 partitions = 2 heads x 64).
  S^T is computed per (i-block 512, j-chunk 128) into a 2-bank PSUM tile
  holding BOTH heads of the pair; one ACT instr exps both heads into a
  bf16 P tile (bf16 moving operand keeps 1 cyc/row even for the 128-wide
  diagonal chunks). V is bf16 with a fused ones column so the PV matmul
  emits y_unnorm and the softmax denominator together; scores are O(5)
  so exp needs no max-subtraction. Softmax renorm: rec = exp(-ln D) on
  ACT (activation tables reordered so Exp and Ln share one table set),
  broadcast across partitions on the idle GpSimd engine, multiplied in
  on DVE. Output collects in one bf16 SBUF tile, stored in 4 big DMAs,
  summed in f32 on the host.
"""

import numpy as np
import ml_dtypes

import concourse.bacc as bacc
import concourse.hw_specs as hw_specs
import concourse.mybir as mybir
import concourse.tile as tile
from concourse.bass_utils import run_bass_kernel_spmd

F32 = mybir.dt.float32
F32R = mybir.dt.float32r
BF16 = mybir.dt.bfloat16
AF = mybir.ActivationFunctionType

T = 2048            # sequence length
D = 1024            # model dim
DH = 64             # head dim
HPC = 4             # heads per core
NCORES = 8
NTT = T // 128      # 16 t-tiles of 128
NDC = D // 128      # 8 d-chunks of 128
NIB = T // 512      # 4 i-blocks of 512
JPB = 512 // 128    # j-chunks per i-block
VW = DH + 2         # v row stride: 64 v + 1 ones + 1 pad (4B alignment)

_TABLES_PATCHED = False


def _patch_act_tables():
    """Prefer natural_log_exp_and_others so Exp and Ln activations share
    one table set (otherwise the per-renorm Ln thrashes ~2.7us reloads)."""
    global _TABLES_PATCHED
    if _TABLES_PATCHED:
        return
    _TABLES_PATCHED = True
    orig = hw_specs.get_activation_tables

    def patched(arch):
        # act_func_set_id is positional (index into act_info.json), so the
        # dict order/size must be preserved. Steer the chooser by removing
        # Exp/Ln from every OTHER set, so both resolve to the shared set.
        tabs = dict(orig(arch))
        pref = "natural_log_exp_and_others"
        if pref in tabs:
            drop = {AF.Exp, AF.Ln}
            tabs = {k: (v if k == pref else set(v) - drop)
                    for k, v in tabs.items()}
        return tabs

    hw_specs.get_activation_tables = patched
    bacc.get_activation_tables = patched


def _build():
    _patch_act_tables()
    nc = bacc.Bacc("TRN2", target_bir_lowering=False, debug=False)

    XT = nc.dram_tensor("XT", [128, NDC, T], F32R, kind="ExternalInput")
    WQ = nc.dram_tensor("WQ", [128, NDC, 256], F32R, kind="ExternalInput")
    WK = nc.dram_tensor("WK", [128, NDC, 256], F32R, kind="ExternalInput")
    WV = nc.dram_tensor("WV", [128, NDC, 256], F32R, kind="ExternalInput")
    WO = nc.dram_tensor("WO", [128, 2, D], F32R, kind="ExternalInput")
    TRI = nc.dram_tensor("TRI", [128, 128], BF16, kind="ExternalInput")
    ONESC = nc.dram_tensor("ONESC", [128, NTT, HPC, 1], BF16, kind="ExternalInput")
    ONES1 = nc.dram_tensor("ONES1", [1, 64], F32R, kind="ExternalInput")
    OUT = nc.dram_tensor("OUT", [128, NTT, D], BF16, kind="ExternalOutput")

    with tile.TileContext(nc) as tc:
        with tc.tile_pool(name="persist", bufs=1) as pp:
            xt = pp.tile([128, NDC, T], F32R, tag="xt")
            wq_sb = pp.tile([128, NDC, 256], F32R, tag="wq")
            wk_sb = pp.tile([128, NDC, 256], F32R, tag="wk")
            wv_sb = pp.tile([128, NDC, 256], F32R, tag="wv")
            wo_sb = pp.tile([128, 2, D], F32R, tag="wo")
            qt = [pp.tile([128, T], F32R, tag=f"qt{p}", name=f"qt{p}")
                  for p in range(2)]
            kt = [pp.tile([128, T], F32R, tag=f"kt{p}", name=f"kt{p}")
                  for p in range(2)]
            vones = pp.tile([128, NTT, HPC, VW], BF16, tag="vones")
            ones1 = pp.tile([1, 64], F32R, tag="ones1")
            ypair = [pp.tile([128, T], F32R, tag=f"yp{p}", name=f"yp{p}")
                     for p in range(2)]
            tri = pp.tile([128, 128], BF16, tag="tri")

            # input DMAs spread across three issue queues so descriptor
            # generation (~0.6-1.3us per dma_start) runs in parallel;
            # within each queue, earliest-needed first.
            nc.sync.dma_start(xt[:, 0, :], XT[:, 0, :])
            nc.sync.dma_start(xt[:, 1, :], XT[:, 1, :])
            nc.sync.dma_start(xt[:, 2, :], XT[:, 2, :])
            nc.sync.dma_start(xt[:, 3, :], XT[:, 3, :])
            nc.scalar.dma_start(wv_sb[:], WV[:])
            nc.scalar.dma_start(xt[:, 4, :], XT[:, 4, :])
            nc.scalar.dma_start(xt[:, 5, :], XT[:, 5, :])
            nc.scalar.dma_start(wq_sb[:], WQ[:])
            nc.gpsimd.dma_start(out=xt[:, 6, :], in_=XT[:, 6, :])
            nc.gpsimd.dma_start(out=xt[:, 7, :], in_=XT[:, 7, :])
            nc.gpsimd.dma_start(out=wk_sb[:], in_=WK[:])
            nc.gpsimd.dma_start(out=wo_sb[:], in_=WO[:])
            nc.gpsimd.dma_start(out=tri[:], in_=TRI[:])
            nc.gpsimd.dma_start(out=vones[:, :, :, DH:DH + 1], in_=ONESC[:])
            nc.gpsimd.dma_start(out=ones1[:], in_=ONES1[:])

            # ---------------- phase A: q/k/v projections ----------------
            with tc.tile_pool(name="psv", bufs=3, space="PSUM") as psv, \
                 tc.tile_pool(name="psqk", bufs=5, space="PSUM") as psqk:
                def v_proj(ti):
                    # v[t, dh] for 4 heads, natural layout, bf16
                    vp = psv.tile([128, 256], F32, tag="vp")
                    for dc in range(NDC):
                        nc.tensor.matmul(
                            vp[:], xt[:, dc, ti * 128:(ti + 1) * 128],
                            wv_sb[:, dc, :],
                            start=(dc == 0), stop=(dc == NDC - 1))
                    nc.vector.tensor_copy(
                        vones[:, ti, :, 0:DH],
                        vp[:].rearrange("p (h d) -> p h d", h=HPC))

                def qk_proj(w_sb, dst, pi):
                    # dc-outer: one LDWEIGHTS per (w, dc) across 4 i-blocks
                    qp = [psqk.tile([128, 512], F32, tag="qkp", name=f"qp{i}")
                          for i in range(NIB)]
                    for dc in range(NDC):
                        for ib in range(NIB):
                            nc.tensor.matmul(
                                qp[ib][:],
                                w_sb[:, dc, pi * 128:(pi + 1) * 128],
                                xt[:, dc, ib * 512:(ib + 1) * 512],
                                start=(dc == 0), stop=(dc == NDC - 1))
                    for ib in range(NIB):
                        nc.vector.tensor_copy(
                            dst[pi][:, ib * 512:(ib + 1) * 512], qp[ib][:])

                # pair-0 attention (ib=0) needs v tiles 0..3 + q/k pair 0;
                # emit those first so phase B starts as early as possible.
                for ti in range(4):
                    v_proj(ti)
                qk_proj(wq_sb, qt, 0)
                qk_proj(wk_sb, kt, 0)
                for ti in range(4, NTT):
                    v_proj(ti)
                qk_proj(wq_sb, qt, 1)
                qk_proj(wk_sb, kt, 1)

            # ---------------- phase B: causal attention ----------------
            with tc.tile_pool(name="phBpt", bufs=3) as pbpt, \
                 tc.tile_pool(name="phBn", bufs=1) as pbn, \
                 tc.tile_pool(name="psBst", bufs=2, space="PSUM") as psbst, \
                 tc.tile_pool(name="psBy", bufs=1, space="PSUM") as psby, \
                 tc.tile_pool(name="psBbc", bufs=2, space="PSUM") as psbbc:
                for pi in range(2):
                    for ib in range(NIB):
                        jlast = JPB * ib + JPB - 1
                        yab = psby.tile([128, 2, 512], F32, tag="yab")
                        for jc in range(jlast + 1):
                            off = 128 * (jc - JPB * ib) if jc >= JPB * ib else 0
                            stab = psbst.tile([128, 2, 512], F32, tag="stab")
                            ptab = pbpt.tile([128, 2, 512], BF16, tag="ptab")
                            js = slice(jc * 128, (jc + 1) * 128)
                            isl = slice(ib * 512 + off, (ib + 1) * 512)
                            nc.tensor.matmul(
                                stab[:, 0, off:512], kt[pi][0:64, js],
                                qt[pi][0:64, isl], start=True, stop=True)
                            nc.tensor.matmul(
                                stab[:, 1, off:512], kt[pi][64:128, js],
                                qt[pi][64:128, isl], start=True, stop=True,
                                tile_position=(64, 0))
                            nc.scalar.activation(
                                ptab[:, :, off:512], stab[:, :, off:512],
                                AF.Exp, scale=0.125)
                            if jc >= JPB * ib:  # diagonal chunk: mask triangle
                                nc.vector.tensor_mul(
                                    ptab[:, 0, off:off + 128],
                                    ptab[:, 0, off:off + 128], tri[:])
                                nc.vector.tensor_mul(
                                    ptab[:, 1, off:off + 128],
                                    ptab[:, 1, off:off + 128], tri[:])
                            for h in range(2):
                                nc.tensor.matmul(
                                    yab[0:65, h, off:512],
                                    vones[:, jc, 2 * pi + h, 0:DH + 1],
                                    ptab[:, h, off:512],
                                    start=(jc == 0), stop=(jc == jlast))
                        # renormalize: y /= denom (row 64); rec = exp(-ln D)
                        # on ACT, partition-broadcast on idle GpSimd.
                        ibs = slice(ib * 512, (ib + 1) * 512)
                        lnd = pbn.tile([1, 2, 512], F32, tag="lnd")
                        rec = pbn.tile([1, 2, 512], F32R, tag="rec")
                        nc.scalar.activation(lnd[:], yab[64:65, :, :], AF.Ln)
                        nc.scalar.activation(rec[:], lnd[:], AF.Exp, scale=-1.0)
                        for h in range(2):
                            bc = psbbc.tile([64, 512], F32, tag="bc")
                            bcs = pbn.tile([64, 512], F32R, tag=f"bcs{h}",
                                           name=f"bcs{h}")
                            nc.tensor.matmul(
                                bc[:], ones1[:], rec[0:1, h, :],
                                start=True, stop=True)
                            nc.vector.tensor_copy(bcs[:], bc[:])
                            nc.vector.tensor_mul(
                                ypair[pi][64 * h:64 * h + 64, ibs],
                                yab[0:64, h, :], bcs[:])

            # ---------------- phase C: out-projection ----------------
            with tc.tile_pool(name="phC", bufs=2) as pc_, \
                 tc.tile_pool(name="psC", bufs=4, space="PSUM") as psc:
                for tg in range(NTT // 4):
                    ostg = pc_.tile([128, 4, D], BF16, tag="ostg")
                    for tq in range(4):
                        ti = 4 * tg + tq
                        for eh in range(2):
                            op = psc.tile([128, 512], F32, tag="op")
                            for pi in range(2):
                                nc.tensor.matmul(
                                    op[:],
                                    ypair[pi][:, ti * 128:(ti + 1) * 128],
                                    wo_sb[:, pi, eh * 512:(eh + 1) * 512],
                                    start=(pi == 0), stop=(pi == 1))
                            nc.vector.tensor_copy(
                                ostg[:, tq, eh * 512:(eh + 1) * 512], op[:])
                    nc.sync.dma_start(OUT[:, 4 * tg:4 * tg + 4, :], ostg[:])

    nc.compile()
    return nc


_NC = None


def build_in_maps(x, w_qkv, w_out):
    x = np.asarray(x, np.float32)
    w_qkv = np.asarray(w_qkv, np.float32)
    w_out = np.asarray(w_out, np.float32)

    tri = np.triu(np.ones((128, 128), np.float32)).astype(
        ml_dtypes.bfloat16)                                # tri[j,i]=1 iff j<=i
    onesc = np.ones((128, NTT, HPC, 1), ml_dtypes.bfloat16)
    ones1 = np.ones((1, 64), np.float32)

    # [d, n] -> [128, d//128, n] with partition p: d = chunk*128 + p
    def dswz(w, dt=np.float32):
        return np.ascontiguousarray(
            w.reshape(NDC, 128, -1).transpose(1, 0, 2)).astype(dt)

    in_maps = []
    for c in range(NCORES):
        b, g = divmod(c, 4)
        cs = slice(g * 256, (g + 1) * 256)
        in_maps.append({
            "XT": dswz(np.ascontiguousarray(x[b].T)),
            "WQ": dswz(np.ascontiguousarray(w_qkv[:, 0:1024][:, cs])),
            "WK": dswz(np.ascontiguousarray(w_qkv[:, 1024:2048][:, cs])),
            "WV": dswz(np.ascontiguousarray(w_qkv[:, 2048:3072][:, cs])),
            "WO": np.ascontiguousarray(
                w_out[g * 256:(g + 1) * 256, :].reshape(2, 128, D)
                .transpose(1, 0, 2)),
            "TRI": tri, "ONESC": onesc, "ONES1": ones1,
        })
    return in_maps


def kernel(x, w_qkv, w_out):
    global _NC
    if _NC is None:
        _NC = _build()

    in_maps = build_in_maps(x, w_qkv, w_out)
    res = run_bass_kernel_spmd(_NC, in_maps, core_ids=list(range(NCORES)))
    # OUT is [128, NTT, D] with row t = ti*128 + p -> unswizzle to [T, D]
    outs = [res.results[c]["OUT"].astype(np.float32)
            .transpose(1, 0, 2).reshape(T, D) for c in range(NCORES)]
    y = np.stack([outs[0] + outs[1] + outs[2] + outs[3],
                  outs[4] + outs[5] + outs[6] + outs[7]], axis=0)
    return y.astype(np.float32)


# revision 27
# speedup vs baseline: 1.0250x; 1.0250x over previous
"""Causal self-attention (b=2, t=2048, d=1024, h=16) on 8 trn2 NeuronCores.

Sharding: core c handles batch c//4 and the 4 heads 4*(c%4)..4*(c%4)+3
(data parallel over batch x tensor parallel over heads). Each core
computes x @ w_qkv for its head-slice, causal attention for its heads,
and a partial out-projection  y_heads @ w_out[head_rows]; the host sums
the 4 partial outputs per batch (the tensor-parallel all-reduce).

Layout/perf notes:
  x is transposed on the HOST (f32) so the kernel does plain contiguous
  DMAs into f32r tiles (no DMA-transpose, no hi/lo bf16 split, no DVE
  merge). Weights are host-swizzled to [128, chunks, n].
  Input DMAs are spread across the Sync/Scalar/GpSimd queues so issue
  (~1.3us each) does not serialize the head of the kernel.
  qT, kT [dh, t] f32r per head-pair (128 partitions = 2 heads x 64).
  S^T is computed per (i-block 512, j-chunk 128) into a 2-bank PSUM tile
  holding BOTH heads of the pair; one ACT instr exps both heads into a
  bf16 P tile (bf16 moving operand keeps 1 cyc/row even for the 128-wide
  diagonal chunks). V is bf16 with a fused ones column so the PV matmul
  emits y_unnorm and the softmax denominator together; scores are O(5)
  so exp needs no max-subtraction. Softmax renorm: rec = exp(-ln D) on
  ACT (activation tables reordered so Exp and Ln share one table set),
  broadcast across partitions on the idle GpSimd engine, multiplied in
  on DVE. Output collects in one bf16 SBUF tile, stored in 4 big DMAs,
  summed in f32 on the host.
"""

import numpy as np
import ml_dtypes

import concourse.bacc as bacc
import concourse.hw_specs as hw_specs
import concourse.mybir as mybir
import concourse.tile as tile
from concourse.bass_utils import run_bass_kernel_spmd

F32 = mybir.dt.float32
F32R = mybir.dt.float32r
BF16 = mybir.dt.bfloat16
AF = mybir.ActivationFunctionType

T = 2048            # sequence length
D = 1024            # model dim
DH = 64             # head dim
HPC = 4             # heads per core
NCORES = 8
NTT = T // 128      # 16 t-tiles of 128
NDC = D // 128      # 8 d-chunks of 128
NIB = T // 512      # 4 i-blocks of 512
JPB = 512 // 128    # j-chunks per i-block
VW = DH + 2         # v row stride: 64 v + 1 ones + 1 pad (4B alignment)

_TABLES_PATCHED = False


def _patch_act_tables():
    """Prefer natural_log_exp_and_others so Exp and Ln activations share
    one table set (otherwise the per-renorm Ln thrashes ~2.7us reloads)."""
    global _TABLES_PATCHED
    if _TABLES_PATCHED:
        return
    _TABLES_PATCHED = True
    orig = hw_specs.get_activation_tables

    def patched(arch):
        # act_func_set_id is positional (index into act_info.json), so the
        # dict order/size must be preserved. Steer the chooser by removing
        # Exp/Ln from every OTHER set, so both resolve to the shared set.
        tabs = dict(orig(arch))
        pref = "natural_log_exp_and_others"
        if pref in tabs:
            drop = {AF.Exp, AF.Ln}
            tabs = {k: (v if k == pref else set(v) - drop)
                    for k, v in tabs.items()}
        return tabs

    hw_specs.get_activation_tables = patched
    bacc.get_activation_tables = patched


def _build():
    _patch_act_tables()
    nc = bacc.Bacc("TRN2", target_bir_lowering=False, debug=False)

    XT = nc.dram_tensor("XT", [128, NDC, T], F32R, kind="ExternalInput")
    WQ = nc.dram_tensor("WQ", [128, NDC, 256], F32R, kind="ExternalInput")
    WK = nc.dram_tensor("WK", [128, NDC, 256], F32R, kind="ExternalInput")
    WV = nc.dram_tensor("WV", [128, NDC, 256], F32R, kind="ExternalInput")
    WO = nc.dram_tensor("WO", [128, 2, D], F32R, kind="ExternalInput")
    TRI = nc.dram_tensor("TRI", [128, 128], BF16, kind="ExternalInput")
    ONESC = nc.dram_tensor("ONESC", [128, NTT, HPC, 1], BF16, kind="ExternalInput")
    ONES1 = nc.dram_tensor("ONES1", [1, 64], F32R, kind="ExternalInput")
    OUT = nc.dram_tensor("OUT", [128, NTT, D], BF16, kind="ExternalOutput")

    with tile.TileContext(nc) as tc:
        with tc.tile_pool(name="persist", bufs=1) as pp:
            xt = pp.tile([128, NDC, T], F32R, tag="xt")
            wq_sb = pp.tile([128, NDC, 256], F32R, tag="wq")
            wk_sb = pp.tile([128, NDC, 256], F32R, tag="wk")
            wv_sb = pp.tile([128, NDC, 256], F32R, tag="wv")
            wo_sb = pp.tile([128, 2, D], F32R, tag="wo")
            qt = [pp.tile([128, T], F32R, tag=f"qt{p}", name=f"qt{p}")
                  for p in range(2)]
            kt = [pp.tile([128, T], F32R, tag=f"kt{p}", name=f"kt{p}")
                  for p in range(2)]
            vones = pp.tile([128, NTT, HPC, VW], BF16, tag="vones")
            ones1 = pp.tile([1, 64], F32R, tag="ones1")
            ypair = [pp.tile([128, T], F32R, tag=f"yp{p}", name=f"yp{p}")
                     for p in range(2)]
            tri = pp.tile([128, 128], BF16, tag="tri")

            # input DMAs spread across three issue queues so descriptor
            # generation (~0.6-1.3us per dma_start) runs in parallel;
            # within each queue, earliest-needed first.
            nc.sync.dma_start(xt[:, 0, :], XT[:, 0, :])
            nc.sync.dma_start(xt[:, 1, :], XT[:, 1, :])
            nc.sync.dma_start(xt[:, 2, :], XT[:, 2, :])
            nc.sync.dma_start(xt[:, 3, :], XT[:, 3, :])
            nc.sync.dma_start(ones1[:], ONES1[:])
            nc.sync.dma_start(tri[:], TRI[:])
            nc.sync.dma_start(vones[:, :, :, DH:DH + 1], ONESC[:])
            nc.scalar.dma_start(wv_sb[:], WV[:])
            nc.scalar.dma_start(wq_sb[:], WQ[:])
            nc.scalar.dma_start(wk_sb[:], WK[:])
            nc.scalar.dma_start(xt[:, 4, :], XT[:, 4, :])
            nc.scalar.dma_start(xt[:, 5, :], XT[:, 5, :])
            nc.scalar.dma_start(xt[:, 6, :], XT[:, 6, :])
            nc.scalar.dma_start(xt[:, 7, :], XT[:, 7, :])
            nc.scalar.dma_start(wo_sb[:], WO[:])

            # ---------------- phase A: q/k/v projections ----------------
            with tc.tile_pool(name="psv", bufs=3, space="PSUM") as psv, \
                 tc.tile_pool(name="psqk", bufs=5, space="PSUM") as psqk:
                def v_proj(ti):
                    # v[t, dh] for 4 heads, natural layout, bf16
                    vp = psv.tile([128, 256], F32, tag="vp")
                    for dc in range(NDC):
                        nc.tensor.matmul(
                            vp[:], xt[:, dc, ti * 128:(ti + 1) * 128],
                            wv_sb[:, dc, :],
                            start=(dc == 0), stop=(dc == NDC - 1))
                    nc.vector.tensor_copy(
                        vones[:, ti, :, 0:DH],
                        vp[:].rearrange("p (h d) -> p h d", h=HPC))

                def qk_proj(w_sb, dst, pi):
                    # dc-outer: one LDWEIGHTS per (w, dc) across 4 i-blocks
                    qp = [psqk.tile([128, 512], F32, tag="qkp", name=f"qp{i}")
                          for i in range(NIB)]
                    for dc in range(NDC):
                        for ib in range(NIB):
                            nc.tensor.matmul(
                                qp[ib][:],
                                w_sb[:, dc, pi * 128:(pi + 1) * 128],
                                xt[:, dc, ib * 512:(ib + 1) * 512],
                                start=(dc == 0), stop=(dc == NDC - 1))
                    for ib in range(NIB):
                        nc.vector.tensor_copy(
                            dst[pi][:, ib * 512:(ib + 1) * 512], qp[ib][:])

                # pair-0 attention (ib=0) needs v tiles 0..3 + q/k pair 0;
                # emit those first so phase B starts as early as possible.
                for ti in range(4):
                    v_proj(ti)
                qk_proj(wq_sb, qt, 0)
                qk_proj(wk_sb, kt, 0)
                for ti in range(4, NTT):
                    v_proj(ti)
                qk_proj(wq_sb, qt, 1)
                qk_proj(wk_sb, kt, 1)

            # ---------------- phase B: causal attention ----------------
            with tc.tile_pool(name="phBpt", bufs=4) as pbpt, \
                 tc.tile_pool(name="phBn", bufs=1) as pbn, \
                 tc.tile_pool(name="psBst", bufs=2, space="PSUM") as psbst, \
                 tc.tile_pool(name="psBy", bufs=2, space="PSUM") as psby:
                def make_renorm(pi, ib, yab):
                    # renormalize: y /= denom (row 64). Emitted DEFERRED --
                    # after the next block's first chunks -- so the PE's
                    # in-order stream has independent S/PV matmuls queued
                    # ahead of the bc matmul that waits on the ACT chain.
                    def renorm():
                        ibs = slice(ib * 512, (ib + 1) * 512)
                        lnd = pbn.tile([1, 2, 512], F32R, tag="lnd",
                                       name="lnd")
                        nc.scalar.activation(lnd[:], yab[64:65, :, :], AF.Ln)
                        for h in range(2):
                            bc = psbst.tile([64, 512], F32, tag="stab",
                                            name=f"bc{h}")
                            bcs = pbn.tile([64, 512], F32R, tag=f"bcs{h}",
                                           name=f"bcs{h}")
                            nc.tensor.matmul(
                                bc[:], ones1[:], lnd[0:1, h, :],
                                start=True, stop=True)
                            nc.scalar.activation(
                                bcs[:], bc[:], AF.Exp, scale=-1.0)
                            nc.vector.tensor_mul(
                                ypair[pi][64 * h:64 * h + 64, ibs],
                                yab[0:64, h, :], bcs[:])
                    return renorm

                pending = None
                for pi in range(2):
                    for ib in range(NIB):
                        jlast = JPB * ib + JPB - 1
                        yab = psby.tile([128, 2, 512], F32, tag="yab")
                        for jc in range(jlast + 1):
                            off = 128 * (jc - JPB * ib) if jc >= JPB * ib else 0
                            stab = psbst.tile([128, 2, 512], F32, tag="stab")
                            ptab = pbpt.tile([128, 2, 512], BF16, tag="ptab")
                            js = slice(jc * 128, (jc + 1) * 128)
                            isl = slice(ib * 512 + off, (ib + 1) * 512)
                            nc.tensor.matmul(
                                stab[:, 0, off:512], kt[pi][0:64, js],
                                qt[pi][0:64, isl], start=True, stop=True)
                            nc.tensor.matmul(
                                stab[:, 1, off:512], kt[pi][64:128, js],
                                qt[pi][64:128, isl], start=True, stop=True,
                                tile_position=(64, 0))
                            nc.scalar.activation(
                                ptab[:, :, off:512], stab[:, :, off:512],
                                AF.Exp, scale=0.125)
                            if jc >= JPB * ib:  # diagonal chunk: mask triangle
                                nc.vector.tensor_mul(
                                    ptab[:, 0, off:off + 128],
                                    ptab[:, 0, off:off + 128], tri[:])
                                nc.vector.tensor_mul(
                                    ptab[:, 1, off:off + 128],
                                    ptab[:, 1, off:off + 128], tri[:])
                            for h in range(2):
                                nc.tensor.matmul(
                                    yab[0:65, h, off:512],
                                    vones[:, jc, 2 * pi + h, 0:DH + 1],
                                    ptab[:, h, off:512],
                                    start=(jc == 0), stop=(jc == jlast))
                            if jc == 1 and pending is not None:
                                pending()
                                pending = None
                        pending = make_renorm(pi, ib, yab)
                if pending is not None:
                    pending()

            # ---------------- phase C: out-projection ----------------
            with tc.tile_pool(name="phC", bufs=2) as pc_, \
                 tc.tile_pool(name="psC", bufs=4, space="PSUM") as psc:
                for tg in range(NTT // 4):
                    ostg = pc_.tile([128, 4, D], BF16, tag="ostg")
                    for tq in range(4):
                        ti = 4 * tg + tq
                        for eh in range(2):
                            op = psc.tile([128, 512], F32, tag="op")
                            for pi in range(2):
                                nc.tensor.matmul(
                                    op[:],
                                    ypair[pi][:, ti * 128:(ti + 1) * 128],
                                    wo_sb[:, pi, eh * 512:(eh + 1) * 512],
                                    start=(pi == 0), stop=(pi == 1))
                            nc.vector.tensor_copy(
                                ostg[:, tq, eh * 512:(eh + 1) * 512], op[:])
                    nc.sync.dma_start(OUT[:, 4 * tg:4 * tg + 4, :], ostg[:])

    nc.compile()
    return nc


_NC = None


def build_in_maps(x, w_qkv, w_out):
    x = np.asarray(x, np.float32)
    w_qkv = np.asarray(w_qkv, np.float32)
    w_out = np.asarray(w_out, np.float32)

    tri = np.triu(np.ones((128, 128), np.float32)).astype(
        ml_dtypes.bfloat16)                                # tri[j,i]=1 iff j<=i
    onesc = np.ones((128, NTT, HPC, 1), ml_dtypes.bfloat16)
    ones1 = np.ones((1, 64), np.float32)

    # [d, n] -> [128, d//128, n] with partition p: d = chunk*128 + p
    def dswz(w, dt=np.float32):
        return np.ascontiguousarray(
            w.reshape(NDC, 128, -1).transpose(1, 0, 2)).astype(dt)

    in_maps = []
    for c in range(NCORES):
        b, g = divmod(c, 4)
        cs = slice(g * 256, (g + 1) * 256)
        in_maps.append({
            "XT": dswz(np.ascontiguousarray(x[b].T)),
            "WQ": dswz(np.ascontiguousarray(w_qkv[:, 0:1024][:, cs])),
            "WK": dswz(np.ascontiguousarray(w_qkv[:, 1024:2048][:, cs])),
            "WV": dswz(np.ascontiguousarray(w_qkv[:, 2048:3072][:, cs])),
            "WO": np.ascontiguousarray(
                w_out[g * 256:(g + 1) * 256, :].reshape(2, 128, D)
                .transpose(1, 0, 2)),
            "TRI": tri, "ONESC": onesc, "ONES1": ones1,
        })
    return in_maps


def kernel(x, w_qkv, w_out):
    global _NC
    if _NC is None:
        _NC = _build()

    in_maps = build_in_maps(x, w_qkv, w_out)
    res = run_bass_kernel_spmd(_NC, in_maps, core_ids=list(range(NCORES)))
    # OUT is [128, NTT, D] with row t = ti*128 + p -> unswizzle to [T, D]
    outs = [res.results[c]["OUT"].astype(np.float32)
            .transpose(1, 0, 2).reshape(T, D) for c in range(NCORES)]
    y = np.stack([outs[0] + outs[1] + outs[2] + outs[3],
                  outs[4] + outs[5] + outs[6] + outs[7]], axis=0)
    return y.astype(np.float32)


# revision 28
# speedup vs baseline: 1.2728x; 1.2418x over previous
"""Causal self-attention (b=2, t=2048, d=1024, h=16) on 8 trn2 NeuronCores.

Sharding: core c handles batch c//4 and the 4 heads 4*(c%4)..4*(c%4)+3
(data parallel over batch x tensor parallel over heads). Each core
computes x @ w_qkv for its head-slice, causal attention for its heads,
and a partial out-projection  y_heads @ w_out[head_rows]; the host sums
the 4 partial outputs per batch (the tensor-parallel all-reduce).

Layout/perf notes:
  x is transposed on the HOST (f32) so the kernel does plain contiguous
  DMAs into f32r tiles (no DMA-transpose, no hi/lo bf16 split, no DVE
  merge). Weights are host-swizzled to [128, chunks, n].
  Input DMAs are spread across the Sync/Scalar/GpSimd queues so issue
  (~1.3us each) does not serialize the head of the kernel.
  qT, kT [dh, t] f32r per head-pair (128 partitions = 2 heads x 64).
  S^T is computed per (i-block 512, j-chunk 128) into a 2-bank PSUM tile
  holding BOTH heads of the pair; one ACT instr exps both heads into a
  bf16 P tile (bf16 moving operand keeps 1 cyc/row even for the 128-wide
  diagonal chunks). V is bf16 with a fused ones column so the PV matmul
  emits y_unnorm and the softmax denominator together; scores are O(5)
  so exp needs no max-subtraction. Softmax renorm: rec = exp(-ln D) on
  ACT (activation tables reordered so Exp and Ln share one table set),
  broadcast across partitions on the idle GpSimd engine, multiplied in
  on DVE. Output collects in one bf16 SBUF tile, stored in 4 big DMAs,
  summed in f32 on the host.
"""

import numpy as np
import ml_dtypes

import concourse.bacc as bacc
import concourse.hw_specs as hw_specs
import concourse.mybir as mybir
import concourse.tile as tile
from concourse.bass_utils import run_bass_kernel_spmd

F32 = mybir.dt.float32
F32R = mybir.dt.float32r
BF16 = mybir.dt.bfloat16
AF = mybir.ActivationFunctionType

T = 2048            # sequence length
D = 1024            # model dim
DH = 64             # head dim
HPC = 4             # heads per core
NCORES = 8
NTT = T // 128      # 16 t-tiles of 128
NDC = D // 128      # 8 d-chunks of 128
NIB = T // 512      # 4 i-blocks of 512
JPB = 512 // 128    # j-chunks per i-block
VW = DH + 2         # v row stride: 64 v + 1 ones + 1 pad (4B alignment)

_TABLES_PATCHED = False


def _patch_act_tables():
    """Prefer natural_log_exp_and_others so Exp and Ln activations share
    one table set (otherwise the per-renorm Ln thrashes ~2.7us reloads)."""
    global _TABLES_PATCHED
    if _TABLES_PATCHED:
        return
    _TABLES_PATCHED = True
    orig = hw_specs.get_activation_tables

    def patched(arch):
        # act_func_set_id is positional (index into act_info.json), so the
        # dict order/size must be preserved. Steer the chooser by removing
        # Exp/Ln from every OTHER set, so both resolve to the shared set.
        tabs = dict(orig(arch))
        pref = "natural_log_exp_and_others"
        if pref in tabs:
            drop = {AF.Exp, AF.Ln}
            tabs = {k: (v if k == pref else set(v) - drop)
                    for k, v in tabs.items()}
        return tabs

    hw_specs.get_activation_tables = patched
    bacc.get_activation_tables = patched


def _build():
    _patch_act_tables()
    nc = bacc.Bacc("TRN2", target_bir_lowering=False, debug=False)

    XT = nc.dram_tensor("XT", [128, NDC, T], BF16, kind="ExternalInput")
    WQ = nc.dram_tensor("WQ", [128, NDC, 256], BF16, kind="ExternalInput")
    WK = nc.dram_tensor("WK", [128, NDC, 256], BF16, kind="ExternalInput")
    WV = nc.dram_tensor("WV", [128, NDC, 256], BF16, kind="ExternalInput")
    WO = nc.dram_tensor("WO", [128, 2, D], F32R, kind="ExternalInput")
    TRI = nc.dram_tensor("TRI", [128, 128], BF16, kind="ExternalInput")
    ONESC = nc.dram_tensor("ONESC", [128, NTT, HPC, 1], BF16, kind="ExternalInput")
    ONES1 = nc.dram_tensor("ONES1", [1, 64], F32R, kind="ExternalInput")
    OUT = nc.dram_tensor("OUT", [128, NTT, D], BF16, kind="ExternalOutput")

    with tile.TileContext(nc) as tc:
        with tc.tile_pool(name="persist", bufs=1) as pp:
            xt = pp.tile([128, NDC, T], BF16, tag="xt")
            wq_sb = pp.tile([128, NDC, 256], BF16, tag="wq")
            wk_sb = pp.tile([128, NDC, 256], BF16, tag="wk")
            wv_sb = pp.tile([128, NDC, 256], BF16, tag="wv")
            wo_sb = pp.tile([128, 2, D], F32R, tag="wo")
            qt = [pp.tile([128, T], F32R, tag=f"qt{p}", name=f"qt{p}")
                  for p in range(2)]
            kt = [pp.tile([128, T], F32R, tag=f"kt{p}", name=f"kt{p}")
                  for p in range(2)]
            vones = pp.tile([128, NTT, HPC, VW], BF16, tag="vones")
            ones1 = pp.tile([1, 64], F32R, tag="ones1")
            ypair = [pp.tile([128, T], F32R, tag=f"yp{p}", name=f"yp{p}")
                     for p in range(2)]
            tri = pp.tile([128, 128], BF16, tag="tri")

            # input DMAs spread across three issue queues so descriptor
            # generation (~0.6-1.3us per dma_start) runs in parallel;
            # within each queue, earliest-needed first.
            nc.sync.dma_start(xt[:, 0, :], XT[:, 0, :])
            nc.sync.dma_start(xt[:, 1, :], XT[:, 1, :])
            nc.sync.dma_start(xt[:, 2, :], XT[:, 2, :])
            nc.sync.dma_start(xt[:, 3, :], XT[:, 3, :])
            nc.sync.dma_start(ones1[:], ONES1[:])
            nc.sync.dma_start(tri[:], TRI[:])
            nc.sync.dma_start(vones[:, :, :, DH:DH + 1], ONESC[:])
            nc.scalar.dma_start(wv_sb[:], WV[:])
            nc.scalar.dma_start(wq_sb[:], WQ[:])
            nc.scalar.dma_start(wk_sb[:], WK[:])
            nc.scalar.dma_start(xt[:, 4, :], XT[:, 4, :])
            nc.scalar.dma_start(xt[:, 5, :], XT[:, 5, :])
            nc.scalar.dma_start(xt[:, 6, :], XT[:, 6, :])
            nc.scalar.dma_start(xt[:, 7, :], XT[:, 7, :])
            nc.scalar.dma_start(wo_sb[:], WO[:])

            # ---------------- phase A: q/k/v projections ----------------
            with tc.tile_pool(name="psv", bufs=3, space="PSUM") as psv, \
                 tc.tile_pool(name="psqk", bufs=5, space="PSUM") as psqk:
                def v_proj(ti):
                    # v[t, dh] for 4 heads, natural layout, bf16
                    vp = psv.tile([128, 256], F32, tag="vp")
                    for dc in range(NDC):
                        nc.tensor.matmul(
                            vp[:], xt[:, dc, ti * 128:(ti + 1) * 128],
                            wv_sb[:, dc, :],
                            start=(dc == 0), stop=(dc == NDC - 1))
                    nc.vector.tensor_copy(
                        vones[:, ti, :, 0:DH],
                        vp[:].rearrange("p (h d) -> p h d", h=HPC))

                def qk_proj(w_sb, dst, pi):
                    # dc-outer: one LDWEIGHTS per (w, dc) across 4 i-blocks
                    qp = [psqk.tile([128, 512], F32, tag="qkp", name=f"qp{i}")
                          for i in range(NIB)]
                    for dc in range(NDC):
                        for ib in range(NIB):
                            nc.tensor.matmul(
                                qp[ib][:],
                                w_sb[:, dc, pi * 128:(pi + 1) * 128],
                                xt[:, dc, ib * 512:(ib + 1) * 512],
                                start=(dc == 0), stop=(dc == NDC - 1))
                    for ib in range(NIB):
                        nc.vector.tensor_copy(
                            dst[pi][:, ib * 512:(ib + 1) * 512], qp[ib][:])

                # pair-0 attention (ib=0) needs v tiles 0..3 + q/k pair 0;
                # emit those first so phase B starts as early as possible.
                for ti in range(4):
                    v_proj(ti)
                qk_proj(wq_sb, qt, 0)
                qk_proj(wk_sb, kt, 0)
                for ti in range(4, NTT):
                    v_proj(ti)
                qk_proj(wq_sb, qt, 1)
                qk_proj(wk_sb, kt, 1)

            # ---------------- phase B: causal attention ----------------
            with tc.tile_pool(name="phBpt", bufs=4) as pbpt, \
                 tc.tile_pool(name="phBn", bufs=1) as pbn, \
                 tc.tile_pool(name="psBst", bufs=2, space="PSUM") as psbst, \
                 tc.tile_pool(name="psBy", bufs=2, space="PSUM") as psby:
                def make_renorm(pi, ib, yab):
                    # renormalize: y /= denom (row 64). Emitted DEFERRED --
                    # after the next block's first chunks -- so the PE's
                    # in-order stream has independent S/PV matmuls queued
                    # ahead of the bc matmul that waits on the ACT chain.
                    def renorm():
                        ibs = slice(ib * 512, (ib + 1) * 512)
                        lnd = pbn.tile([1, 2, 512], F32R, tag="lnd",
                                       name="lnd")
                        nc.scalar.activation(lnd[:], yab[64:65, :, :], AF.Ln)
                        for h in range(2):
                            bc = psbst.tile([64, 512], F32, tag="stab",
                                            name=f"bc{h}")
                            bcs = pbn.tile([64, 512], F32R, tag=f"bcs{h}",
                                           name=f"bcs{h}")
                            nc.tensor.matmul(
                                bc[:], ones1[:], lnd[0:1, h, :],
                                start=True, stop=True)
                            nc.scalar.activation(
                                bcs[:], bc[:], AF.Exp, scale=-1.0)
                            nc.vector.tensor_mul(
                                ypair[pi][64 * h:64 * h + 64, ibs],
                                yab[0:64, h, :], bcs[:])
                    return renorm

                pending = None
                for pi in range(2):
                    for ib in range(NIB):
                        jlast = JPB * ib + JPB - 1
                        yab = psby.tile([128, 2, 512], F32, tag="yab")
                        for jc in range(jlast + 1):
                            off = 128 * (jc - JPB * ib) if jc >= JPB * ib else 0
                            stab = psbst.tile([128, 2, 512], F32, tag="stab")
                            ptab = pbpt.tile([128, 2, 512], BF16, tag="ptab")
                            js = slice(jc * 128, (jc + 1) * 128)
                            isl = slice(ib * 512 + off, (ib + 1) * 512)
                            nc.tensor.matmul(
                                stab[:, 0, off:512], kt[pi][0:64, js],
                                qt[pi][0:64, isl], start=True, stop=True)
                            nc.tensor.matmul(
                                stab[:, 1, off:512], kt[pi][64:128, js],
                                qt[pi][64:128, isl], start=True, stop=True,
                                tile_position=(64, 0))
                            nc.scalar.activation(
                                ptab[:, :, off:512], stab[:, :, off:512],
                                AF.Exp, scale=0.125)
                            if jc >= JPB * ib:  # diagonal chunk: mask triangle
                                nc.vector.tensor_mul(
                                    ptab[:, 0, off:off + 128],
                                    ptab[:, 0, off:off + 128], tri[:])
                                nc.vector.tensor_mul(
                                    ptab[:, 1, off:off + 128],
                                    ptab[:, 1, off:off + 128], tri[:])
                            for h in range(2):
                                nc.tensor.matmul(
                                    yab[0:65, h, off:512],
                                    vones[:, jc, 2 * pi + h, 0:DH + 1],
                                    ptab[:, h, off:512],
                                    start=(jc == 0), stop=(jc == jlast))
                            if jc == 1 and pending is not None:
                                pending()
                                pending = None
                        pending = make_renorm(pi, ib, yab)
                if pending is not None:
                    pending()

            # ---------------- phase C: out-projection ----------------
            with tc.tile_pool(name="phC", bufs=2) as pc_, \
                 tc.tile_pool(name="psC", bufs=4, space="PSUM") as psc:
                for tg in range(NTT // 4):
                    ostg = pc_.tile([128, 4, D], BF16, tag="ostg")
                    for tq in range(4):
                        ti = 4 * tg + tq
                        for eh in range(2):
                            op = psc.tile([128, 512], F32, tag="op")
                            for pi in range(2):
                                nc.tensor.matmul(
                                    op[:],
                                    ypair[pi][:, ti * 128:(ti + 1) * 128],
                                    wo_sb[:, pi, eh * 512:(eh + 1) * 512],
                                    start=(pi == 0), stop=(pi == 1))
                            nc.vector.tensor_copy(
                                ostg[:, tq, eh * 512:(eh + 1) * 512], op[:])
                    nc.sync.dma_start(OUT[:, 4 * tg:4 * tg + 4, :], ostg[:])

    nc.compile()
    return nc


_NC = None


def build_in_maps(x, w_qkv, w_out):
    x = np.asarray(x, np.float32)
    w_qkv = np.asarray(w_qkv, np.float32)
    w_out = np.asarray(w_out, np.float32)

    tri = np.triu(np.ones((128, 128), np.float32)).astype(
        ml_dtypes.bfloat16)                                # tri[j,i]=1 iff j<=i
    onesc = np.ones((128, NTT, HPC, 1), ml_dtypes.bfloat16)
    ones1 = np.ones((1, 64), np.float32)

    # [d, n] -> [128, d//128, n] with partition p: d = chunk*128 + p
    def dswz(w, dt=np.float32):
        return np.ascontiguousarray(
            w.reshape(NDC, 128, -1).transpose(1, 0, 2)).astype(dt)

    in_maps = []
    for c in range(NCORES):
        b, g = divmod(c, 4)
        cs = slice(g * 256, (g + 1) * 256)
        in_maps.append({
            "XT": dswz(np.ascontiguousarray(x[b].T), ml_dtypes.bfloat16),
            "WQ": dswz(np.ascontiguousarray(w_qkv[:, 0:1024][:, cs]),
                       ml_dtypes.bfloat16),
            "WK": dswz(np.ascontiguousarray(w_qkv[:, 1024:2048][:, cs]),
                       ml_dtypes.bfloat16),
            "WV": dswz(np.ascontiguousarray(w_qkv[:, 2048:3072][:, cs]),
                       ml_dtypes.bfloat16),
            "WO": np.ascontiguousarray(
                w_out[g * 256:(g + 1) * 256, :].reshape(2, 128, D)
                .transpose(1, 0, 2)),
            "TRI": tri, "ONESC": onesc, "ONES1": ones1,
        })
    return in_maps


def kernel(x, w_qkv, w_out):
    global _NC
    if _NC is None:
        _NC = _build()

    in_maps = build_in_maps(x, w_qkv, w_out)
    res = run_bass_kernel_spmd(_NC, in_maps, core_ids=list(range(NCORES)))
    # OUT is [128, NTT, D] with row t = ti*128 + p -> unswizzle to [T, D]
    outs = [res.results[c]["OUT"].astype(np.float32)
            .transpose(1, 0, 2).reshape(T, D) for c in range(NCORES)]
    y = np.stack([outs[0] + outs[1] + outs[2] + outs[3],
                  outs[4] + outs[5] + outs[6] + outs[7]], axis=0)
    return y.astype(np.float32)


# revision 29
# speedup vs baseline: 1.5292x; 1.2014x over previous
"""Causal self-attention (b=2, t=2048, d=1024, h=16) on 8 trn2 NeuronCores.

Sharding: core c handles batch c//4 and the 4 heads 4*(c%4)..4*(c%4)+3
(data parallel over batch x tensor parallel over heads). Each core
computes x @ w_qkv for its head-slice, causal attention for its heads,
and a partial out-projection  y_heads @ w_out[head_rows]; the host sums
the 4 partial outputs per batch (the tensor-parallel all-reduce).

Layout/perf notes:
  x is transposed on the HOST (f32) so the kernel does plain contiguous
  DMAs into f32r tiles (no DMA-transpose, no hi/lo bf16 split, no DVE
  merge). Weights are host-swizzled to [128, chunks, n].
  Input DMAs are spread across the Sync/Scalar/GpSimd queues so issue
  (~1.3us each) does not serialize the head of the kernel.
  qT, kT [dh, t] f32r per head-pair (128 partitions = 2 heads x 64).
  S^T is computed per (i-block 512, j-chunk 128) into a 2-bank PSUM tile
  holding BOTH heads of the pair; one ACT instr exps both heads into a
  bf16 P tile (bf16 moving operand keeps 1 cyc/row even for the 128-wide
  diagonal chunks). V is bf16 with a fused ones column so the PV matmul
  emits y_unnorm and the softmax denominator together; scores are O(5)
  so exp needs no max-subtraction. Softmax renorm: rec = exp(-ln D) on
  ACT (activation tables reordered so Exp and Ln share one table set),
  broadcast across partitions on the idle GpSimd engine, multiplied in
  on DVE. Output collects in one bf16 SBUF tile, stored in 4 big DMAs,
  summed in f32 on the host.
"""

import numpy as np
import ml_dtypes

import concourse.bacc as bacc
import concourse.hw_specs as hw_specs
import concourse.mybir as mybir
import concourse.tile as tile
from concourse.bass_utils import run_bass_kernel_spmd

F32 = mybir.dt.float32
F32R = mybir.dt.float32r
BF16 = mybir.dt.bfloat16
AF = mybir.ActivationFunctionType

T = 2048            # sequence length
D = 1024            # model dim
DH = 64             # head dim
HPC = 4             # heads per core
NCORES = 8
NTT = T // 128      # 16 t-tiles of 128
NDC = D // 128      # 8 d-chunks of 128
NIB = T // 512      # 4 i-blocks of 512
JPB = 512 // 128    # j-chunks per i-block
VW = DH + 2         # v row stride: 64 v + 1 ones + 1 pad (4B alignment)

_TABLES_PATCHED = False


def _patch_act_tables():
    """Prefer natural_log_exp_and_others so Exp and Ln activations share
    one table set (otherwise the per-renorm Ln thrashes ~2.7us reloads)."""
    global _TABLES_PATCHED
    if _TABLES_PATCHED:
        return
    _TABLES_PATCHED = True
    orig = hw_specs.get_activation_tables

    def patched(arch):
        # act_func_set_id is positional (index into act_info.json), so the
        # dict order/size must be preserved. Steer the chooser by removing
        # Exp/Ln from every OTHER set, so both resolve to the shared set.
        tabs = dict(orig(arch))
        pref = "natural_log_exp_and_others"
        if pref in tabs:
            drop = {AF.Exp, AF.Ln}
            tabs = {k: (v if k == pref else set(v) - drop)
                    for k, v in tabs.items()}
        return tabs

    hw_specs.get_activation_tables = patched
    bacc.get_activation_tables = patched


def _build():
    _patch_act_tables()
    nc = bacc.Bacc("TRN2", target_bir_lowering=False, debug=False)

    XT = nc.dram_tensor("XT", [128, NDC, T], BF16, kind="ExternalInput")
    WQ = nc.dram_tensor("WQ", [128, NDC, 256], BF16, kind="ExternalInput")
    WK = nc.dram_tensor("WK", [128, NDC, 256], BF16, kind="ExternalInput")
    WV = nc.dram_tensor("WV", [128, NDC, 256], BF16, kind="ExternalInput")
    WO = nc.dram_tensor("WO", [128, 2, D], F32R, kind="ExternalInput")
    TRI = nc.dram_tensor("TRI", [128, 128], BF16, kind="ExternalInput")
    ONESC = nc.dram_tensor("ONESC", [128, NTT, HPC, 1], BF16, kind="ExternalInput")
    ONES1 = nc.dram_tensor("ONES1", [1, 64], F32R, kind="ExternalInput")
    OUT = nc.dram_tensor("OUT", [128, NTT, D], BF16, kind="ExternalOutput")

    with tile.TileContext(nc) as tc:
        with tc.tile_pool(name="persist", bufs=1) as pp:
            xt = pp.tile([128, NDC, T], BF16, tag="xt")
            wq_sb = pp.tile([128, NDC, 256], BF16, tag="wq")
            wk_sb = pp.tile([128, NDC, 256], BF16, tag="wk")
            wv_sb = pp.tile([128, NDC, 256], BF16, tag="wv")
            wo_sb = pp.tile([128, 2, D], F32R, tag="wo")
            qt = [pp.tile([128, T], F32R, tag=f"qt{p}", name=f"qt{p}")
                  for p in range(2)]
            kt = [pp.tile([128, T], F32R, tag=f"kt{p}", name=f"kt{p}")
                  for p in range(2)]
            vones = pp.tile([128, NTT, HPC, VW], BF16, tag="vones")
            ones1 = pp.tile([1, 64], F32R, tag="ones1")
            ypair = [pp.tile([128, T], F32R, tag=f"yp{p}", name=f"yp{p}")
                     for p in range(2)]
            tri = pp.tile([128, 128], BF16, tag="tri")

            # input DMAs spread across three issue queues so descriptor
            # generation (~0.6-1.3us per dma_start) runs in parallel;
            # within each queue, earliest-needed first.
            nc.sync.dma_start(xt[:, 0, :], XT[:, 0, :])
            nc.sync.dma_start(xt[:, 1, :], XT[:, 1, :])
            nc.sync.dma_start(xt[:, 2, :], XT[:, 2, :])
            nc.sync.dma_start(xt[:, 3, :], XT[:, 3, :])
            nc.sync.dma_start(ones1[:], ONES1[:])
            nc.sync.dma_start(tri[:], TRI[:])
            nc.sync.dma_start(vones[:, :, :, DH:DH + 1], ONESC[:])
            nc.scalar.dma_start(wv_sb[:], WV[:])
            nc.scalar.dma_start(wq_sb[:], WQ[:])
            nc.scalar.dma_start(wk_sb[:], WK[:])
            nc.scalar.dma_start(xt[:, 4, :], XT[:, 4, :])
            nc.scalar.dma_start(xt[:, 5, :], XT[:, 5, :])
            nc.scalar.dma_start(xt[:, 6, :], XT[:, 6, :])
            nc.scalar.dma_start(xt[:, 7, :], XT[:, 7, :])
            nc.scalar.dma_start(wo_sb[:], WO[:])

            # ---------------- phase A: q/k/v projections ----------------
            with tc.tile_pool(name="psv", bufs=3, space="PSUM") as psv, \
                 tc.tile_pool(name="psqk", bufs=5, space="PSUM") as psqk:
                def v_proj(ti):
                    # v[t, dh] for 4 heads, natural layout, bf16
                    vp = psv.tile([128, 256], F32, tag="vp")
                    for dc in range(NDC):
                        nc.tensor.matmul(
                            vp[:], xt[:, dc, ti * 128:(ti + 1) * 128],
                            wv_sb[:, dc, :],
                            start=(dc == 0), stop=(dc == NDC - 1))
                    nc.vector.tensor_copy(
                        vones[:, ti, :, 0:DH],
                        vp[:].rearrange("p (h d) -> p h d", h=HPC))

                def qk_proj(w_sb, dst, pi):
                    # dc-outer: one LDWEIGHTS per (w, dc) across 4 i-blocks
                    qp = [psqk.tile([128, 512], F32, tag="qkp", name=f"qp{i}")
                          for i in range(NIB)]
                    for dc in range(NDC):
                        for ib in range(NIB):
                            nc.tensor.matmul(
                                qp[ib][:],
                                w_sb[:, dc, pi * 128:(pi + 1) * 128],
                                xt[:, dc, ib * 512:(ib + 1) * 512],
                                start=(dc == 0), stop=(dc == NDC - 1))
                    for ib in range(NIB):
                        nc.vector.tensor_copy(
                            dst[pi][:, ib * 512:(ib + 1) * 512], qp[ib][:])

                # Only the work phase B(pair 0, ib 0) needs runs ahead
                # of attention: v tiles 0..3 and q/k for pair 0. The rest
                # (v 4..15, q/k pair 1) is interleaved into phase B at
                # block boundaries, borrowing its PSUM slots, so it fills
                # the ACT-paced stall slack there.
                for ti in range(4):
                    v_proj(ti)
                qk_proj(wq_sb, qt, 0)
                qk_proj(wk_sb, kt, 0)

            # ------------- phase B: causal attention (+ interleaves) -------
            # Per chunk the PE stream is software-pipelined: S(jc+1) is
            # emitted BEFORE PV(jc) so the in-order PE has independent work
            # while ACT runs exp(jc). Renorms are deferred into the next
            # block; leftover projections (v 4..15, q/k pair 1) borrow
            # stab/yab PSUM slots at block boundaries; phase-C groups are
            # emitted under B(pair 1) as their ypair inputs complete.
            with tc.tile_pool(name="phBpt", bufs=4) as pbpt, \
                 tc.tile_pool(name="phBn", bufs=1) as pbn, \
                 tc.tile_pool(name="phC", bufs=2) as pc_, \
                 tc.tile_pool(name="psBst", bufs=2, space="PSUM") as psbst, \
                 tc.tile_pool(name="psBy", bufs=2, space="PSUM") as psby:
                def v_proj_b(ti):
                    vp = psbst.tile([128, 256], F32, tag="stab", name="vpb")
                    for dc in range(NDC):
                        nc.tensor.matmul(
                            vp[:], xt[:, dc, ti * 128:(ti + 1) * 128],
                            wv_sb[:, dc, :],
                            start=(dc == 0), stop=(dc == NDC - 1))
                    nc.vector.tensor_copy(
                        vones[:, ti, :, 0:DH],
                        vp[:].rearrange("p (h d) -> p h d", h=HPC))

                def qk1_chain(w_sb, dst, ib):
                    qp = psby.tile([128, 512], F32, tag="yab", name="qp1")
                    for dc in range(NDC):
                        nc.tensor.matmul(
                            qp[:], w_sb[:, dc, 128:256],
                            xt[:, dc, ib * 512:(ib + 1) * 512],
                            start=(dc == 0), stop=(dc == NDC - 1))
                    nc.vector.tensor_copy(
                        dst[1][:, ib * 512:(ib + 1) * 512], qp[:])

                def make_renorm(pi, ib, yab):
                    def renorm():
                        ibs = slice(ib * 512, (ib + 1) * 512)
                        lnd = pbn.tile([1, 2, 512], F32R, tag="lnd",
                                       name="lnd")
                        nc.scalar.activation(lnd[:], yab[64:65, :, :], AF.Ln)
                        for h in range(2):
                            bc = psbst.tile([64, 512], F32, tag="stab",
                                            name=f"bc{h}")
                            bcs = pbn.tile([64, 512], F32R, tag=f"bcs{h}",
                                           name=f"bcs{h}")
                            nc.tensor.matmul(
                                bc[:], ones1[:], lnd[0:1, h, :],
                                start=True, stop=True)
                            nc.scalar.activation(
                                bcs[:], bc[:], AF.Exp, scale=-1.0)
                            nc.vector.tensor_mul(
                                ypair[pi][64 * h:64 * h + 64, ibs],
                                yab[0:64, h, :], bcs[:])
                    return renorm

                ostg = {}

                def c_pair(g, tq):
                    # out-projection for t-tile 4g+tq (both 512-halves)
                    if tq == 0:
                        ostg[g] = pc_.tile([128, 4, D], BF16, tag="ostg",
                                           name=f"ostg{g}")
                    ti = 4 * g + tq
                    for eh in range(2):
                        op = psby.tile([128, 512], F32, tag="yab",
                                       name=f"op{eh}")
                        for pi in range(2):
                            nc.tensor.matmul(
                                op[:], ypair[pi][:, ti * 128:(ti + 1) * 128],
                                wo_sb[:, pi, eh * 512:(eh + 1) * 512],
                                start=(pi == 0), stop=(pi == 1))
                        nc.vector.tensor_copy(
                            ostg[g][:, tq, eh * 512:(eh + 1) * 512], op[:])
                    if tq == 3:
                        nc.sync.dma_start(
                            OUT[:, 4 * g:4 * g + 4, :], ostg[g][:])

                pending = None
                for pi in range(2):
                    for ib in range(NIB):
                        jlast = JPB * ib + JPB - 1
                        yab = psby.tile([128, 2, 512], F32, tag="yab")
                        prev_pv = None
                        cgrp = None
                        for jc in range(jlast + 1):
                            off = 128 * (jc - JPB * ib) if jc >= JPB * ib else 0
                            stab = psbst.tile([128, 2, 512], F32, tag="stab")
                            ptab = pbpt.tile([128, 2, 512], BF16, tag="ptab")
                            js = slice(jc * 128, (jc + 1) * 128)
                            isl = slice(ib * 512 + off, (ib + 1) * 512)
                            nc.tensor.matmul(
                                stab[:, 0, off:512], kt[pi][0:64, js],
                                qt[pi][0:64, isl], start=True, stop=True)
                            nc.tensor.matmul(
                                stab[:, 1, off:512], kt[pi][64:128, js],
                                qt[pi][64:128, isl], start=True, stop=True,
                                tile_position=(64, 0))
                            nc.scalar.activation(
                                ptab[:, :, off:512], stab[:, :, off:512],
                                AF.Exp, scale=0.125)
                            if jc >= JPB * ib:  # diagonal chunk: mask tri
                                nc.vector.tensor_mul(
                                    ptab[:, 0, off:off + 128],
                                    ptab[:, 0, off:off + 128], tri[:])
                                nc.vector.tensor_mul(
                                    ptab[:, 1, off:off + 128],
                                    ptab[:, 1, off:off + 128], tri[:])
                            if prev_pv is not None:
                                prev_pv()
                            if jc == 1 and pending is not None:
                                pending()
                                pending = None
                                if pi == 1 and ib >= 1:
                                    cgrp = ib - 1  # its ypair is complete now
                            if cgrp is not None and 2 <= jc <= 5:
                                c_pair(cgrp, jc - 2)

                            def mk_pv(yab, jc, off, pi, ptab):
                                def pv():
                                    for h in range(2):
                                        nc.tensor.matmul(
                                            yab[0:65, h, off:512],
                                            vones[:, jc, 2 * pi + h, 0:DH + 1],
                                            ptab[:, h, off:512],
                                            start=(jc == 0), stop=(jc == jlast))
                                return pv
                            prev_pv = mk_pv(yab, jc, off, pi, ptab)
                        prev_pv()
                        pending = make_renorm(pi, ib, yab)
                        if pi == 0:
                            # block-boundary fillers: v tiles for the NEXT
                            # block, then 2 q/k pair-1 chains
                            if ib < NIB - 1:
                                for k in range(4):
                                    v_proj_b(4 * (ib + 1) + k)
                            qk1_chain(wq_sb, qt, ib)
                            qk1_chain(wk_sb, kt, ib)
                if pending is not None:
                    pending()
                for tq in range(4):
                    c_pair(NIB - 1, tq)

    nc.compile()
    return nc


_NC = None


def build_in_maps(x, w_qkv, w_out):
    x = np.asarray(x, np.float32)
    w_qkv = np.asarray(w_qkv, np.float32)
    w_out = np.asarray(w_out, np.float32)

    tri = np.triu(np.ones((128, 128), np.float32)).astype(
        ml_dtypes.bfloat16)                                # tri[j,i]=1 iff j<=i
    onesc = np.ones((128, NTT, HPC, 1), ml_dtypes.bfloat16)
    ones1 = np.ones((1, 64), np.float32)

    # [d, n] -> [128, d//128, n] with partition p: d = chunk*128 + p
    def dswz(w, dt=np.float32):
        return np.ascontiguousarray(
            w.reshape(NDC, 128, -1).transpose(1, 0, 2)).astype(dt)

    in_maps = []
    for c in range(NCORES):
        b, g = divmod(c, 4)
        cs = slice(g * 256, (g + 1) * 256)
        in_maps.append({
            "XT": dswz(np.ascontiguousarray(x[b].T), ml_dtypes.bfloat16),
            "WQ": dswz(np.ascontiguousarray(w_qkv[:, 0:1024][:, cs]),
                       ml_dtypes.bfloat16),
            "WK": dswz(np.ascontiguousarray(w_qkv[:, 1024:2048][:, cs]),
                       ml_dtypes.bfloat16),
            "WV": dswz(np.ascontiguousarray(w_qkv[:, 2048:3072][:, cs]),
                       ml_dtypes.bfloat16),
            "WO": np.ascontiguousarray(
                w_out[g * 256:(g + 1) * 256, :].reshape(2, 128, D)
                .transpose(1, 0, 2)),
            "TRI": tri, "ONESC": onesc, "ONES1": ones1,
        })
    return in_maps


def kernel(x, w_qkv, w_out):
    global _NC
    if _NC is None:
        _NC = _build()

    in_maps = build_in_maps(x, w_qkv, w_out)
    res = run_bass_kernel_spmd(_NC, in_maps, core_ids=list(range(NCORES)))
    # OUT is [128, NTT, D] with row t = ti*128 + p -> unswizzle to [T, D]
    outs = [res.results[c]["OUT"].astype(np.float32)
            .transpose(1, 0, 2).reshape(T, D) for c in range(NCORES)]
    y = np.stack([outs[0] + outs[1] + outs[2] + outs[3],
                  outs[4] + outs[5] + outs[6] + outs[7]], axis=0)
    return y.astype(np.float32)
